# revision 18
# baseline (speedup 1.0000x reference)
"""CrossTransformer kernel for 8 Trainium2 NeuronCores (v2, bf16 matmuls).

Sharding: data-parallel over batch B=64 -> 8 batches/core. The two
BatchNorm reductions ((B,S) channel stats and (B,H) energy stats) are
cross-core AllReduces of the per-core partial sums (4KB / 7.7KB).

Math restructuring (fewer FLOPs than the naive projection path):
  energy2[b,h,s,t] = sum_c A[b,h,s,c] * ctx[b,t,c],   A = q2 @ Wk  (per head)
  Z[b,h,s,c]      = sum_t attn2[b,h,s,t] * ctx[b,t,c]
  out2            = (Z @ Wv.T per head, concat) @ Wout.T
This contracts ctx directly (31 rows/head instead of 64 k/v channels),
skipping the k2/v2 projections entirely.

energy2 is computed transposed ([t, (h,s)] per batch) so the softmax
denominators come from ones-matmuls on the PE and attn2.T feeds the Z
matmul without a transpose. The host rearranges the returned tensor.

All matmul operands are bf16 (host pre-casts ctx + weights); every
accumulation, normalization statistic, residual, and both outputs stay
fp32. ctx.T comes from the xbar DMA-transpose (2-byte dtype), so the
PE runs no ctx transposes at all.
"""

import sys
import numpy as np

sys.path.insert(0, "/opt/trn_rl_repo")

P = 128
B, S, CTX, C, H = 64, 31, 2048, 512, 8
DH = C // H
NCORES = 8
BL = B // NCORES          # local batches
SP = 32                   # padded S
RWS = BL * SP             # padded local rows = 256
CT = C // P               # 4 c-tiles
TT = CTX // P             # 16 t-tiles
MID = 2 * C
MT = MID // P             # 8 mid-tiles
EPS = 1e-5
INV_SCALE = 1.0 / float(np.sqrt(np.float32(C)))
NROWS = B * S             # 1984 rows globally for BN1
NBH = B * H               # 512 groups globally for BN2


def build_bass():
    import concourse.bass as bass
    import concourse.tile as tile
    from concourse import bacc, mybir

    F32 = mybir.dt.float32
    BF = mybir.dt.bfloat16
    AF = mybir.ActivationFunctionType
    OP = mybir.AluOpType

    nc = bacc.Bacc(None, target_bir_lowering=False, num_devices=NCORES)

    def din(name, shape, dt=None):
        return nc.dram_tensor(name, list(shape), dt or F32, kind="ExternalInput")

    q_in = din("q_in", [BL, S, C])
    qall_in = din("qall_in", [2048, C])  # host-padded with zero rows
    ctx_in = din("ctx_in", [BL, CTX, C], BF)
    w_saqT = din("w_saqT", [C, C], BF)
    w_sakT = din("w_sakT", [C, C], BF)
    w_savT = din("w_savT", [C, C], BF)
    w_saoT = din("w_saoT", [C, C], BF)
    w_caqT = din("w_caqT", [C, C], BF)
    w_cak = din("w_cak", [C, C], BF)      # natural [hdh, c]
    w_cavT = din("w_cavT", [C, C], BF)    # [c, hdh]
    w_caoT = din("w_caoT", [C, C], BF)    # [hdh, o]
    w_ff1T = din("w_ff1T", [C, MID], BF)
    w_ff2T = din("w_ff2T", [MID, C], BF)
    bn1_g = din("bn1_g", [P, CT])
    bn1_b = din("bn1_b", [P, CT])
    bn2_g = din("bn2_g", [P, S])
    bn2_b = din("bn2_b", [P, S])
    lncag = din("lncag", [P, CT])
    lncab = din("lncab", [P, CT])
    saob = din("saob", [P, CT])
    caob = din("caob", [P, CT])
    ln1g = din("ln1g", [P, CT])
    ln1b = din("ln1b", [P, CT])
    ln2g = din("ln2g", [P, MT])
    ln2b = din("ln2b", [P, MT])
    ident_in = din("ident", [P, P])
    identb_in = din("identb", [P, P], BF)
    ones_in = din("ones", [P, 1])
    onesb_in = din("onesb", [P, 1], BF)
    sel_in = din("sel", [P, S])

    out_x = nc.dram_tensor("out_x", [BL, S, C], F32, kind="ExternalOutput")
    e_out = nc.dram_tensor("e_out", [BL, CTX, H * S], F32, kind="ExternalOutput")

    with tile.TileContext(nc) as tc:
        with (
            tc.tile_pool(name="consts", bufs=1) as consts,
            tc.tile_pool(name="wts", bufs=4) as wts,
            tc.tile_pool(name="big", bufs=2) as big,
            tc.tile_pool(name="ctxp", bufs=3) as ctxp,
            tc.tile_pool(name="ctxtp", bufs=2) as ctxtp,
            tc.tile_pool(name="ptp", bufs=1) as ptp,
            tc.tile_pool(name="ebp", bufs=3) as ebp,
            tc.tile_pool(name="acts", bufs=3) as acts,
            tc.tile_pool(name="cmacts", bufs=5) as cmacts,
            tc.tile_pool(name="tiny", bufs=3) as tiny,
            tc.tile_pool(name="ptr", bufs=3, space="PSUM") as ptr,
            tc.tile_pool(name="pmm", bufs=3, space="PSUM") as pmm,
            tc.tile_pool(name="pz", bufs=2, space="PSUM") as pz,
            tc.tile_pool(name="dram", bufs=2, space="DRAM") as dram,
        ):
            # ---------------- collective warmup (absorbs first-call cost) ----
            warm_i = dram.tile([1, P], F32)
            warm_o = dram.tile([1, P], F32)
            wtile = tiny.tile([1, P], F32, tag="warm", bufs=1)
            nc.vector.memset(wtile[:], 0.0)
            nc.gpsimd.dma_start(warm_i[:], wtile[:])
            nc.gpsimd.collective_compute(
                "AllReduce", OP.add,
                replica_groups=[list(range(NCORES))],
                ins=[warm_i[:].opt()], outs=[warm_o[:].opt()],
            )

            # ---------------- constants ----------------
            ident = consts.tile([P, P], F32)
            nc.sync.dma_start(ident[:], ident_in[:])
            identb = consts.tile([P, P], BF)
            nc.sync.dma_start(identb[:], identb_in[:])
            ones = consts.tile([P, 1], F32)
            nc.sync.dma_start(ones[:], ones_in[:])
            onesb = consts.tile([P, 1], BF)
            nc.sync.dma_start(onesb[:], onesb_in[:])
            sel = consts.tile([P, S], F32)
            nc.sync.dma_start(sel[:], sel_in[:])
            eps_t = consts.tile([P, 1], F32)
            nc.vector.memset(eps_t[:], EPS)

            def cvec(dr, w):
                t = consts.tile([P, w], F32, name=dr.name + "_sb")
                nc.sync.dma_start(t[:], dr[:])
                return t

            bn1g_t = cvec(bn1_g, CT)
            bn1b_t = cvec(bn1_b, CT)
            lncag_t = cvec(lncag, CT)
            lncab_t = cvec(lncab, CT)
            saob_t = cvec(saob, CT)
            caob_t = cvec(caob, CT)
            ln1g_t = cvec(ln1g, CT)
            ln1b_t = cvec(ln1b, CT)
            ln2g_t = cvec(ln2g, MT)
            ln2b_t = cvec(ln2b, MT)
            bn2g_t = cvec(bn2_g, S)
            bn2b_t = cvec(bn2_b, S)

            def wload(dr, kt, width):
                t = wts.tile([P, kt, width], BF, name=dr.name + "_sb", tag="w512")
                nc.gpsimd.dma_start(
                    t[:], dr[:].rearrange("(kt p) o -> p kt o", p=P)
                )
                return t

            # transpose helper -> psum tile (caller consumes psum directly)
            def tr_ps(src_ap, pin, n):
                dt = src_ap.dtype
                ps = ptr.tile([P, P], dt, tag="tr", name="tps")
                idt = (identb if dt == BF else ident)[:pin, :pin]
                nc.tensor.transpose(ps[:n, :pin], src_ap, idt)
                return ps

            # ---------------- queries load + BN1 partials ----------------
            qrm = acts.tile([P, 2, C], F32, tag="rm512")  # rows (4b x 32)
            nc.any.memzero(qrm[:])
            for b in range(BL):
                nc.gpsimd.dma_start(
                    qrm[(b % 4) * SP:(b % 4) * SP + S, b // 4, :], q_in[b]
                )
            # full queries (host zero-padded to 2048 rows), local BN1 stats
            ps_s1 = ptr.tile([1, C], F32, tag="tr", name="ps_s1")
            ps_s2 = ptr.tile([1, C], F32, tag="tr", name="ps_s2")
            for chunk in range(4):
                qc = acts.tile([P, 4, C], F32, tag="qfl", bufs=2, name="qc")
                nc.gpsimd.dma_start(
                    qc[:],
                    qall_in[chunk * 4 * P:(chunk + 1) * 4 * P, :]
                    .rearrange("(n p) c -> p n c", p=P))
                for i in range(4):
                    n = chunk * 4 + i
                    sq = acts.tile([P, C], F32, tag="qsq", bufs=2, name="sq")
                    nc.vector.tensor_mul(sq[:], qc[:, i], qc[:, i])
                    nc.tensor.matmul(ps_s1[:], ones[:], qc[:, i],
                                     start=(n == 0), stop=(n == 15))
                    nc.tensor.matmul(ps_s2[:], ones[:], sq[:],
                                     start=(n == 0), stop=(n == 15))
            st1 = tiny.tile([1, 2, C], F32, tag="st1", bufs=1)
            nc.vector.tensor_copy(st1[:, 0, :], ps_s1[:])
            nc.vector.tensor_copy(st1[:, 1, :], ps_s2[:])
            stats1 = tiny.tile([P, CT, 2], F32, tag="stats1")
            # bounce through DRAM to re-lay [1, 2C] row onto [p, ct, 2]
            bb1 = dram.tile([1, 2 * C], F32)
            nc.sync.dma_start(bb1[:], st1[:].rearrange("a b c -> a (b c)"))
            for j in range(2):
                nc.sync.dma_start(
                    stats1[:, :, j],
                    bb1[0, j * C:(j + 1) * C].rearrange("(ct p) -> p ct", p=P)
                )
            mu1 = tiny.tile([P, CT], F32, tag="mu1")
            nc.scalar.mul(mu1[:], stats1[:, :, 0], 1.0 / NROWS)
            ex2 = tiny.tile([P, CT], F32, tag="ex2")
            nc.scalar.mul(ex2[:], stats1[:, :, 1], 1.0 / NROWS)
            var1 = tiny.tile([P, CT], F32, tag="var1")
            nc.vector.tensor_mul(var1[:], mu1[:], mu1[:])
            nc.vector.tensor_sub(var1[:], ex2[:], var1[:])
            rstd1 = tiny.tile([P, CT], F32, tag="rstd1")
            nc.scalar.activation(rstd1[:], var1[:], AF.Sqrt,
                                 bias=eps_t[:], scale=1.0)
            nc.vector.reciprocal(rstd1[:], rstd1[:])
            s1 = tiny.tile([P, CT], F32, tag="s1")
            nc.vector.tensor_mul(s1[:], rstd1[:], bn1g_t[:])
            t1 = tiny.tile([P, CT], F32, tag="t1")
            nc.vector.tensor_mul(t1[:], mu1[:], s1[:])
            nc.vector.tensor_sub(t1[:], bn1b_t[:], t1[:])

            # ---------------- xnT (bf16, fused BN1 affine) ----------------
            xnT = cmacts.tile([P, CT, RWS], BF, tag="cmb")
            for rt in range(2):
                for ck in range(CT):
                    ps = tr_ps(qrm[:, rt, ck * P:(ck + 1) * P], P, P)
                    nc.vector.tensor_scalar(
                        xnT[:, ck, rt * P:(rt + 1) * P], ps[:],
                        scalar1=s1[:, ck:ck + 1], scalar2=t1[:, ck:ck + 1],
                        op0=OP.mult, op1=OP.add)

            # ---------------- SA projections (bf16, channel-major) --------
            w_q = wload(w_saqT, CT, C)
            w_k = wload(w_sakT, CT, C)
            w_v = wload(w_savT, CT, C)

            def proj(wt, src, kt_n, name):
                out = cmacts.tile([P, CT, RWS], BF, tag="cmb", name=name)
                for ot in range(CT):
                    ps = pmm.tile([P, RWS], F32, tag="mm", name="ps_" + name)
                    for kt in range(kt_n):
                        nc.tensor.matmul(
                            ps[:], wt[:, kt, ot * P:(ot + 1) * P], src[:, kt],
                            start=(kt == 0), stop=(kt == kt_n - 1))
                    nc.any.tensor_copy(out[:, ot], ps[:])
                return out

            qcm = proj(w_q, xnT, CT, "qcm")
            kcm = proj(w_k, xnT, CT, "kcm")
            vcm = proj(w_v, xnT, CT, "vcm")

            # ---------------- SA energy per (b,h) ----------------
            e_rm = acts.tile([P, 16, S], F32, tag="ebh", bufs=2)
            nc.any.memzero(e_rm[:])
            for b in range(BL):
                for h in range(H):
                    g = (b * H + h) // 4
                    r0 = ((b * H + h) % 4) * SP
                    off = (h % 2) * DH
                    ht = h // 2
                    pse = ptr.tile([S, S], F32, tag="tr", name="ps_esa")
                    nc.tensor.matmul(
                        pse[:],
                        qcm[off:off + DH, ht, b * SP:b * SP + S],
                        kcm[off:off + DH, ht, b * SP:b * SP + S],
                        start=True, stop=True,
                        tile_position=(off, 0))
                    nc.any.tensor_copy(e_rm[r0:r0 + S, g, :], pse[:])

            # ---------------- BN2 partials + AllReduce ----------------
            esq = acts.tile([P, 16, S], F32, tag="ebh", bufs=2)
            for g in range(16):
                nc.vector.tensor_mul(esq[:, g], e_rm[:, g], e_rm[:, g])
            ps_b1 = ptr.tile([S, S], F32, tag="tr", name="ps_b1")
            ps_b2 = ptr.tile([S, S], F32, tag="tr", name="ps_b2")
            for g in range(16):
                nc.tensor.matmul(ps_b1[:], sel[:], e_rm[:, g],
                                 start=(g == 0), stop=(g == 15))
            for g in range(16):
                nc.tensor.matmul(ps_b2[:], sel[:], esq[:, g],
                                 start=(g == 0), stop=(g == 15))
            st2 = tiny.tile([S, 2, S], F32, tag="st2", bufs=1)
            nc.any.tensor_copy(st2[:, 0, :], ps_b1[:])
            nc.any.tensor_copy(st2[:, 1, :], ps_b2[:])
            ar2_i = dram.tile([S, 2 * S], F32)
            ar2_o = dram.tile([S, 2 * S], F32)
            nc.sync.dma_start(ar2_i[:], st2[:].rearrange("a b c -> a (b c)"))
            nc.gpsimd.collective_compute(
                "AllReduce", OP.add,
                replica_groups=[list(range(NCORES))],
                ins=[ar2_i[:].opt()], outs=[ar2_o[:].opt()],
            )
            stats2 = tiny.tile([P, 2, S], F32, tag="stats2")
            nc.any.memzero(stats2[:])
            for rep in range(4):
                nc.sync.dma_start(
                    stats2[rep * SP:rep * SP + S],
                    ar2_o[:].rearrange("s (j sp) -> s j sp", j=2))
            mu2 = tiny.tile([P, S], F32, tag="mu2")
            nc.scalar.mul(mu2[:], stats2[:, 0], 1.0 / NBH)
            ex22 = tiny.tile([P, S], F32, tag="ex22")
            nc.scalar.mul(ex22[:], stats2[:, 1], 1.0 / NBH)
            var2 = tiny.tile([P, S], F32, tag="var2")
            nc.vector.tensor_mul(var2[:], mu2[:], mu2[:])
            nc.vector.tensor_sub(var2[:], ex22[:], var2[:])
            rstd2 = tiny.tile([P, S], F32, tag="rstd2")
            nc.scalar.activation(rstd2[:], var2[:], AF.Sqrt,
                                 bias=eps_t[:], scale=1.0)
            nc.vector.reciprocal(rstd2[:], rstd2[:])
            sc2 = tiny.tile([P, S], F32, tag="sc2")
            nc.vector.tensor_mul(sc2[:], rstd2[:], bn2g_t[:])
            off2 = tiny.tile([P, S], F32, tag="off2")
            nc.vector.tensor_mul(off2[:], mu2[:], sc2[:])
            nc.vector.tensor_sub(off2[:], bn2b_t[:], off2[:])

            # normalize + softmax (over s', free dim); attn cast to bf16
            attn_bf = acts.tile([P, 16, S], BF, tag="abh", bufs=1)
            rs_sa = tiny.tile([P, 16], F32, tag="rs_sa")
            for g in range(16):
                nc.vector.tensor_mul(e_rm[:, g], e_rm[:, g], sc2[:])
                nc.vector.tensor_add(e_rm[:, g], e_rm[:, g], off2[:])
                nc.scalar.activation(e_rm[:, g], e_rm[:, g], AF.Exp,
                                     bias=0.0, scale=INV_SCALE)
                nc.vector.reduce_sum(rs_sa[:, g:g + 1], e_rm[:, g],
                                     axis=mybir.AxisListType.X)
            nc.vector.reciprocal(rs_sa[:], rs_sa[:])
            for g in range(16):
                nc.vector.tensor_scalar_mul(attn_bf[:, g], e_rm[:, g],
                                            rs_sa[:, g:g + 1])

            # vT (bf16, row-major)
            vT = acts.tile([P, 2, C], BF, tag="rmb512", bufs=2, name="vT")
            for rt in range(2):
                for ot in range(CT):
                    ps = tr_ps(vcm[:, ot, rt * P:(rt + 1) * P], P, P)
                    nc.any.tensor_copy(vT[:, rt, ot * P:(ot + 1) * P], ps[:])

            # attn.T then attn @ v -> o row-major (fp32)
            o_rm = acts.tile([P, 2, C], F32, tag="rm512", name="o_rm")
            nc.any.memzero(o_rm[:])
            for g in range(16):
                b = g // 2
                po = (b % 4) * SP
                at_t = acts.tile([P, P], BF, tag="atb", bufs=4, name="at_t")
                ps = tr_ps(attn_bf[:, g, :], P, S)
                nc.any.tensor_copy(at_t[po:po + S, :], ps[:S, :])
                for j in range(4):
                    h = (g * 4 + j) % H
                    pso = ptr.tile([S, DH], F32, tag="tr", name="ps_o")
                    nc.tensor.matmul(
                        pso[:],
                        at_t[po:po + S, j * SP:j * SP + S],
                        vT[po:po + S, b // 4, h * DH:(h + 1) * DH],
                        start=True, stop=True,
                        tile_position=(po, 0))
                    nc.any.tensor_copy(
                        o_rm[po:po + S, b // 4, h * DH:(h + 1) * DH], pso[:])

            # ---------------- SA out proj + residual ----------------
            w_o = wload(w_saoT, CT, C)
            oT = cmacts.tile([P, CT, RWS], BF, tag="cmb", name="oT")
            for rt in range(2):
                for ck in range(CT):
                    ps = tr_ps(o_rm[:, rt, ck * P:(ck + 1) * P], P, P)
                    nc.any.tensor_copy(oT[:, ck, rt * P:(rt + 1) * P], ps[:])
            xo_cm = cmacts.tile([P, CT, RWS], F32, tag="cm32", bufs=2,
                                name="xo_cm")
            for ot in range(CT):
                ps = pmm.tile([P, RWS], F32, tag="mm", name="ps_xo")
                for kt in range(CT):
                    nc.tensor.matmul(
                        ps[:], w_o[:, kt, ot * P:(ot + 1) * P], oT[:, kt],
                        start=(kt == 0), stop=(kt == CT - 1))
                nc.vector.tensor_scalar_add(
                    xo_cm[:, ot], ps[:], saob_t[:, ot:ot + 1])
            xsa = acts.tile([P, 2, C], F32, tag="rm512", name="xsa")
            for rt in range(2):
                for ck in range(CT):
                    ps = tr_ps(xo_cm[:, ck, rt * P:(rt + 1) * P], P, P)
                    nc.vector.tensor_add(xsa[:, rt, ck * P:(ck + 1) * P],
                                         ps[:], qrm[:, rt, ck * P:(ck + 1) * P])

            # ---------------- CA: LN(x_sa) -> q2 -> A ----------------
            def ln_rows(src, width, nst, name, inplace=None):
                """row-wise layernorm core (x-mu)*rstd, no affine (fp32)"""
                if inplace is not None:
                    out = inplace
                else:
                    out = acts.tile([P, 2, width], F32,
                                    tag=("rm512" if width == C else "rm1024"),
                                    bufs=(None if width == C else 1),
                                    name=name)
                for rt in range(2):
                    stt = tiny.tile([P, nst, 6], F32, tag="bnst")
                    for j in range(nst):
                        nc.vector.bn_stats(stt[:, j],
                                           src[:, rt, j * C:(j + 1) * C])
                    mv = tiny.tile([P, 2], F32, tag="bnmv")
                    nc.vector.bn_aggr(mv[:], stt[:])
                    rst = tiny.tile([P, 1], F32, tag="bnrs")
                    nc.scalar.activation(rst[:], mv[:, 1:2], AF.Sqrt,
                                         bias=eps_t[:], scale=1.0)
                    nc.vector.reciprocal(rst[:], rst[:])
                    nc.vector.tensor_scalar(
                        out[:, rt], src[:, rt],
                        scalar1=mv[:, 0:1], scalar2=rst[:],
                        op0=OP.subtract, op1=OP.mult)
                return out

            qn = ln_rows(xsa, C, 1, "qn")
            qnT = cmacts.tile([P, CT, RWS], BF, tag="cmb", name="qnT")
            for rt in range(2):
                for ck in range(CT):
                    ps = tr_ps(qn[:, rt, ck * P:(ck + 1) * P], P, P)
                    nc.vector.tensor_scalar(
                        qnT[:, ck, rt * P:(rt + 1) * P], ps[:],
                        scalar1=lncag_t[:, ck:ck + 1],
                        scalar2=lncab_t[:, ck:ck + 1],
                        op0=OP.mult, op1=OP.add)
            w_q2 = wload(w_caqT, CT, C)
            q2cm = proj(w_q2, qnT, CT, "q2cm")

            # A_bf[c, (ct,h,rows)] bf16
            w_k2 = wload(w_cak, CT, C)
            A_bf = big.tile([P, CT, H, RWS], BF, tag="big32", name="A_bf")
            for h in range(H):
                off = (h % 2) * DH
                ht = h // 2
                for ck in range(CT):
                    ps = pmm.tile([P, RWS], F32, tag="mm", name="ps_a")
                    nc.tensor.matmul(
                        ps[:],
                        w_k2[off:off + DH, ht, ck * P:(ck + 1) * P],
                        q2cm[off:off + DH, ht, :],
                        start=True, stop=True,
                        tile_position=(off, 0))
                    nc.any.tensor_copy(A_bf[:, ck, h, :], ps[:])

            # ---------------- main ctx loop ----------------
            ZT = big.tile([P, CT, H, RWS], BF, tag="big32", name="ZT")
            nc.any.memzero(ZT[:])
            for b in range(BL):
                # ctx.T for the whole batch via xbar DMA transpose
                cT = ctxtp.tile([P, CT, CTX], BF, tag="ct", name="cT")
                for ck in range(CT):
                    nc.sync.dma_start_transpose(
                        cT[:, ck, :], ctx_in[b, :, ck * P:(ck + 1) * P])
                p_t = ptp.tile([P, TT, H * S], BF, tag="pt", name="p_t")
                ps_z = [pz.tile([124, C], F32, tag="z", name=f"ps_z{half}")
                        for half in range(2)]
                rsum = tiny.tile([1, H * S], F32, tag="rsum", bufs=2)
                for quarter in range(4):
                    cnat = ctxp.tile([P, 4, C], BF, tag="cn", name="cnat")
                    nc.gpsimd.dma_start(
                        cnat[:],
                        ctx_in[b, quarter * 4 * P:(quarter + 1) * 4 * P, :]
                        .rearrange("(i p) c -> p i c", p=P))
                    e_buf = ebp.tile([P, 4, H, S], F32, tag="eb",
                                     name="e_buf")
                    ps_row = ptr.tile([1, H * S], F32, tag="tr",
                                      name="ps_row")
                    for i in range(4):
                        tt = quarter * 4 + i
                        ps_e = pmm.tile([P, RWS], F32, tag="mm", name="ps_e")
                        for ck in range(CT):
                            nc.tensor.matmul(
                                ps_e[:],
                                cT[:, ck, tt * P:(tt + 1) * P],
                                A_bf[:, ck, :, b * SP:(b + 1) * SP],
                                start=(ck == 0), stop=(ck == CT - 1))
                        pse_v = ps_e[:].rearrange("p (h s) -> p h s", h=H)
                        nc.vector.tensor_copy(e_buf[:, i], pse_v[:, :, :S])
                        nc.scalar.activation(
                            p_t[:, tt].rearrange("p (h s) -> p h s", h=H),
                            pse_v[:, :, :S],
                            AF.Exp, bias=0.0, scale=INV_SCALE)
                        nc.tensor.matmul(
                            ps_row[:], onesb[:], p_t[:, tt, :],
                            start=(i == 0), stop=(i == 3))
                        for half in range(2):
                            nc.tensor.matmul(
                                ps_z[half],
                                p_t[:, tt, half * 124:(half + 1) * 124],
                                cnat[:, i],
                                start=(tt == 0), stop=(tt == TT - 1))
                    if quarter == 0:
                        nc.vector.tensor_copy(rsum[:], ps_row[:])
                    else:
                        nc.vector.tensor_add(rsum[:], rsum[:], ps_row[:])
                    nc.gpsimd.dma_start(
                        e_out[b, quarter * 4 * P:(quarter + 1) * 4 * P, :]
                        .rearrange("(a p) x -> p a x", p=P),
                        e_buf[:].rearrange("p a h s -> p a (h s)"))
                # softmax denominators -> [124, 2] reciprocals
                rr = tiny.tile([124, 2], F32, tag="rr", bufs=2)
                for half in range(2):
                    pst = ptr.tile([124, 1], F32, tag="tr", name="ps_t")
                    nc.tensor.matmul(
                        pst[:], rsum[:1, half * 124:(half + 1) * 124],
                        ident[:1, :1], start=True, stop=True)
                    nc.any.tensor_copy(rr[:, half:half + 1], pst[:])
                rinv = tiny.tile([124, 2], F32, tag="rinv", bufs=2)
                nc.vector.reciprocal(rinv[:], rr[:])
                for half in range(2):
                    z_bf = acts.tile([124, C], BF, tag="zsb", bufs=2,
                                     name="z_bf")
                    nc.vector.tensor_scalar_mul(z_bf[:], ps_z[half][:],
                                                rinv[:, half:half + 1])
                    for ck in range(CT):
                        ps = ptr.tile([P, P], BF, tag="tr", name="tps_z")
                        nc.tensor.transpose(ps[:, :124],
                                            z_bf[:, ck * P:(ck + 1) * P],
                                            identb[:124, :124])
                        nc.vector.tensor_copy(
                            ZT[:, ck, half * 4:(half + 1) * 4,
                               b * SP:b * SP + S],
                            ps[:, :124].rearrange("p (h s) -> p h s", s=S))

            # ---------------- o2 / out2 / residual ----------------
            w_v2 = wload(w_cavT, CT, C)
            o2T = cmacts.tile([P, CT, RWS], BF, tag="cmb", name="o2T")
            nc.any.memzero(o2T[:])
            for h in range(H):
                off = (h % 2) * DH
                ht = h // 2
                ps = pmm.tile([P, RWS], F32, tag="mm", name="ps_o2")
                for ck in range(CT):
                    nc.tensor.matmul(
                        ps[:DH, :],
                        w_v2[:, ck, h * DH:(h + 1) * DH],
                        ZT[:, ck, h, :],
                        start=(ck == 0), stop=(ck == CT - 1))
                nc.any.tensor_copy(o2T[off:off + DH, ht, :], ps[:DH, :])
            w_o2 = wload(w_caoT, CT, C)
            xca = acts.tile([P, 2, C], F32, tag="rm512", name="xca")
            out2cm = cmacts.tile([P, CT, RWS], F32, tag="cm32", bufs=2,
                                 name="out2cm")
            for ot in range(CT):
                ps = pmm.tile([P, RWS], F32, tag="mm", name="ps_out2")
                for kt in range(CT):
                    nc.tensor.matmul(
                        ps[:], w_o2[:, kt, ot * P:(ot + 1) * P], o2T[:, kt],
                        start=(kt == 0), stop=(kt == CT - 1))
                nc.vector.tensor_scalar_add(
                    out2cm[:, ot], ps[:], caob_t[:, ot:ot + 1])
            for rt in range(2):
                for ck in range(CT):
                    ps = tr_ps(out2cm[:, ck, rt * P:(rt + 1) * P], P, P)
                    nc.vector.tensor_add(xca[:, rt, ck * P:(ck + 1) * P],
                                         ps[:], xsa[:, rt, ck * P:(ck + 1) * P])

            # ---------------- FFN ----------------
            h1 = ln_rows(xca, C, 1, "h1")
            h1T = cmacts.tile([P, CT, RWS], BF, tag="cmb", name="h1T")
            for rt in range(2):
                for ck in range(CT):
                    ps = tr_ps(h1[:, rt, ck * P:(ck + 1) * P], P, P)
                    nc.scalar.activation(
                        h1T[:, ck, rt * P:(rt + 1) * P], ps[:], AF.Silu,
                        bias=ln1b_t[:, ck:ck + 1], scale=ln1g_t[:, ck:ck + 1])
            midcm = cmacts.tile([P, MT, RWS], F32, tag="cm32x8", bufs=1,
                                name="midcm")
            for hf in range(2):
                w_f1 = wts.tile([P, CT, C], BF, tag="w512", name=f"w_f1{hf}")
                nc.gpsimd.dma_start(
                    w_f1[:],
                    w_ff1T[:, hf * C:(hf + 1) * C]
                    .rearrange("(kt p) o -> p kt o", p=P))
                for mt_l in range(CT):
                    mt = hf * CT + mt_l
                    ps = pmm.tile([P, RWS], F32, tag="mm", name="ps_mid")
                    for kt in range(CT):
                        nc.tensor.matmul(
                            ps[:], w_f1[:, kt, mt_l * P:(mt_l + 1) * P],
                            h1T[:, kt],
                            start=(kt == 0), stop=(kt == CT - 1))
                    nc.any.tensor_copy(midcm[:, mt], ps[:])
            h2 = acts.tile([P, 2, MID], F32, tag="rm1024", bufs=1, name="h2")
            for rt in range(2):
                for mk in range(MT):
                    ps = tr_ps(midcm[:, mk, rt * P:(rt + 1) * P], P, P)
                    nc.any.tensor_copy(h2[:, rt, mk * P:(mk + 1) * P], ps[:])
            h2n = ln_rows(h2, MID, 2, "h2n", inplace=h2)
            h2T = cmacts.tile([P, MT, RWS], BF, tag="cmbx8", bufs=1,
                              name="h2T")
            for rt in range(2):
                for mk in range(MT):
                    ps = tr_ps(h2n[:, rt, mk * P:(mk + 1) * P], P, P)
                    nc.scalar.activation(
                        h2T[:, mk, rt * P:(rt + 1) * P], ps[:], AF.Silu,
                        bias=ln2b_t[:, mk:mk + 1], scale=ln2g_t[:, mk:mk + 1])
            outx = acts.tile([P, 2, C], F32, tag="rm512", name="outx")
            ffcm = cmacts.tile([P, CT, RWS], F32, tag="cm32", bufs=2,
                               name="ffcm")
            w_f2h = []
            for hf in range(2):
                t = wts.tile([P, CT, C], BF, tag="w512", name=f"w_f2{hf}")
                nc.gpsimd.dma_start(
                    t[:],
                    w_ff2T[hf * C:(hf + 1) * C, :]
                    .rearrange("(kt p) o -> p kt o", p=P))
                w_f2h.append(t)
            for ot in range(CT):
                ps = pmm.tile([P, RWS], F32, tag="mm", name="ps_ff")
                for kt in range(MT):
                    nc.tensor.matmul(
                        ps[:],
                        w_f2h[kt // CT][:, kt % CT, ot * P:(ot + 1) * P],
                        h2T[:, kt],
                        start=(kt == 0), stop=(kt == MT - 1))
                nc.any.tensor_copy(ffcm[:, ot], ps[:])
            for rt in range(2):
                for ck in range(CT):
                    ps = tr_ps(ffcm[:, ck, rt * P:(rt + 1) * P], P, P)
                    nc.vector.tensor_add(outx[:, rt, ck * P:(ck + 1) * P],
                                         ps[:], xca[:, rt, ck * P:(ck + 1) * P])
            for b in range(BL):
                nc.gpsimd.dma_start(
                    out_x[b], outx[(b % 4) * SP:(b % 4) * SP + S, b // 4, :])

    nc.compile()
    return nc


def host_inputs(inputs):
    """Build per-core input maps from full inputs."""
    import ml_dtypes
    f = np.float32
    bf = ml_dtypes.bfloat16
    q = np.asarray(inputs["queries"], f)
    ctx = np.asarray(inputs["contexts"], f)

    def cm(v):  # [C] -> [P, C//P] channel-major per-partition layout
        v = np.asarray(v, f).reshape(-1, P)
        return np.ascontiguousarray(v.T)

    def tb(v):  # transpose + bf16
        return np.ascontiguousarray(np.asarray(v, f).T).astype(bf)

    shared = {
        "w_saqT": tb(inputs["sa_q_w"]),
        "w_sakT": tb(inputs["sa_k_w"]),
        "w_savT": tb(inputs["sa_v_w"]),
        "w_saoT": tb(inputs["sa_out_w"]),
        "w_caqT": tb(inputs["ca_q_w"]),
        "w_cak": np.asarray(inputs["ca_k_w"], f).astype(bf),
        "w_cavT": tb(inputs["ca_v_w"]),
        "w_caoT": tb(inputs["ca_out_w"]),
        "w_ff1T": tb(inputs["ff_w1"]),
        "w_ff2T": tb(inputs["ff_w2"]),
        "bn1_g": cm(inputs["sa_bn_g"]),
        "bn1_b": cm(inputs["sa_bn_b"]),
        "bn2_g": np.tile(np.vstack([np.asarray(inputs["sa_pre_g"], f)
                                    .reshape(S, S), np.zeros((1, S), f)]), (4, 1)),
        "bn2_b": np.tile(np.vstack([np.asarray(inputs["sa_pre_b"], f)
                                    .reshape(S, S), np.zeros((1, S), f)]), (4, 1)),
        "lncag": cm(inputs["ca_ln_g"]),
        "lncab": cm(inputs["ca_ln_b"]),
        "saob": cm(inputs["sa_out_b"]),
        "caob": cm(inputs["ca_out_b"]),
        "ln1g": cm(inputs["ff_ln1_g"]),
        "ln1b": cm(inputs["ff_ln1_b"]),
        "ln2g": cm(inputs["ff_ln2_g"]),
        "ln2b": cm(inputs["ff_ln2_b"]),
        "ident": np.eye(P, dtype=f),
        "identb": np.eye(P, dtype=f).astype(bf),
        "ones": np.ones((P, 1), f),
        "onesb": np.ones((P, 1), f).astype(bf),
        "sel": np.tile(np.vstack([np.eye(S, dtype=f),
                                  np.zeros((1, S), f)]), (4, 1)),
    }
    ctx_bf = ctx.astype(bf)
    q_flat = np.zeros((2048, C), f)
    q_flat[:B * S] = q.reshape(B * S, C)
    in_maps = []
    for i in range(NCORES):
        m = dict(shared)
        m["q_in"] = np.ascontiguousarray(q[i * BL:(i + 1) * BL])
        m["qall_in"] = q_flat
        m["ctx_in"] = np.ascontiguousarray(ctx_bf[i * BL:(i + 1) * BL])
        in_maps.append(m)
    return in_maps


_CACHE = {}


def _get_nc():
    if "nc" not in _CACHE:
        _CACHE["nc"] = build_bass()
    return _CACHE["nc"]


def run(inputs, trace=False, **kw):
    from concourse.bass_utils import run_bass_kernel_spmd

    nc = _get_nc()
    in_maps = host_inputs(inputs)
    res = run_bass_kernel_spmd(nc, in_maps, core_ids=list(range(NCORES)),
                               trace=trace, **kw)
    outs = []
    es = []
    for i in range(NCORES):
        outs.append(res.results[i]["out_x"])
        es.append(res.results[i]["e_out"])
    out = np.concatenate(outs, axis=0).astype(np.float32)
    e_t = np.concatenate(es, axis=0)  # [B, CTX, H*S]
    energy2 = np.ascontiguousarray(
        e_t.reshape(B, CTX, H, S).transpose(0, 2, 3, 1)).astype(np.float32)
    return (out, energy2), res


def kernel(**inputs):
    (out, energy2), _ = run(inputs)
    return out, energy2


# revision 19
# speedup vs baseline: 1.0250x; 1.0250x over previous
"""CrossTransformer kernel for 8 Trainium2 NeuronCores (v2, bf16 matmuls).

Sharding: data-parallel over batch B=64 -> 8 batches/core. The two
BatchNorm reductions ((B,S) channel stats and (B,H) energy stats) are
cross-core AllReduces of the per-core partial sums (4KB / 7.7KB).

Math restructuring (fewer FLOPs than the naive projection path):
  energy2[b,h,s,t] = sum_c A[b,h,s,c] * ctx[b,t,c],   A = q2 @ Wk  (per head)
  Z[b,h,s,c]      = sum_t attn2[b,h,s,t] * ctx[b,t,c]
  out2            = (Z @ Wv.T per head, concat) @ Wout.T
This contracts ctx directly (31 rows/head instead of 64 k/v channels),
skipping the k2/v2 projections entirely.

energy2 is computed transposed ([t, (h,s)] per batch) so the softmax
denominators come from ones-matmuls on the PE and attn2.T feeds the Z
matmul without a transpose. The host rearranges the returned tensor.

All matmul operands are bf16 (host pre-casts ctx + weights); every
accumulation, normalization statistic, residual, and both outputs stay
fp32. ctx.T comes from the xbar DMA-transpose (2-byte dtype), so the
PE runs no ctx transposes at all.
"""

import sys
import numpy as np

sys.path.insert(0, "/opt/trn_rl_repo")

P = 128
B, S, CTX, C, H = 64, 31, 2048, 512, 8
DH = C // H
NCORES = 8
BL = B // NCORES          # local batches
SP = 32                   # padded S
RWS = BL * SP             # padded local rows = 256
CT = C // P               # 4 c-tiles
TT = CTX // P             # 16 t-tiles
MID = 2 * C
MT = MID // P             # 8 mid-tiles
EPS = 1e-5
INV_SCALE = 1.0 / float(np.sqrt(np.float32(C)))
NROWS = B * S             # 1984 rows globally for BN1
NBH = B * H               # 512 groups globally for BN2


def build_bass():
    import concourse.bass as bass
    import concourse.tile as tile
    from concourse import bacc, mybir

    F32 = mybir.dt.float32
    BF = mybir.dt.bfloat16
    AF = mybir.ActivationFunctionType
    OP = mybir.AluOpType

    nc = bacc.Bacc(None, target_bir_lowering=False, num_devices=NCORES)

    def din(name, shape, dt=None):
        return nc.dram_tensor(name, list(shape), dt or F32, kind="ExternalInput")

    q_in = din("q_in", [BL, S, C])
    qall_in = din("qall_in", [2048, C])  # host-padded with zero rows
    ctx_in = din("ctx_in", [BL, CTX, C], BF)
    w_saqT = din("w_saqT", [C, C], BF)
    w_sakT = din("w_sakT", [C, C], BF)
    w_savT = din("w_savT", [C, C], BF)
    w_saoT = din("w_saoT", [C, C], BF)
    w_caqT = din("w_caqT", [C, C], BF)
    w_cak = din("w_cak", [C, C], BF)      # natural [hdh, c]
    w_cavT = din("w_cavT", [C, C], BF)    # [c, hdh]
    w_caoT = din("w_caoT", [C, C], BF)    # [hdh, o]
    w_ff1T = din("w_ff1T", [C, MID], BF)
    w_ff2T = din("w_ff2T", [MID, C], BF)
    bn1_g = din("bn1_g", [P, CT])
    bn1_b = din("bn1_b", [P, CT])
    bn2_g = din("bn2_g", [P, S])
    bn2_b = din("bn2_b", [P, S])
    lncag = din("lncag", [P, CT])
    lncab = din("lncab", [P, CT])
    saob = din("saob", [P, CT])
    caob = din("caob", [P, CT])
    ln1g = din("ln1g", [P, CT])
    ln1b = din("ln1b", [P, CT])
    ln2g = din("ln2g", [P, MT])
    ln2b = din("ln2b", [P, MT])
    ident_in = din("ident", [P, P])
    identb_in = din("identb", [P, P], BF)
    ones_in = din("ones", [P, 1])
    onesb_in = din("onesb", [P, 1], BF)
    sel_in = din("sel", [P, S])

    out_x = nc.dram_tensor("out_x", [BL, S, C], F32, kind="ExternalOutput")
    e_out = nc.dram_tensor("e_out", [BL, CTX, H * S], F32, kind="ExternalOutput")

    with tile.TileContext(nc) as tc:
        with (
            tc.tile_pool(name="consts", bufs=1) as consts,
            tc.tile_pool(name="wts", bufs=4) as wts,
            tc.tile_pool(name="big", bufs=2) as big,
            tc.tile_pool(name="ctxp", bufs=3) as ctxp,
            tc.tile_pool(name="ctxtp", bufs=2) as ctxtp,
            tc.tile_pool(name="ptp", bufs=1) as ptp,
            tc.tile_pool(name="ebp", bufs=3) as ebp,
            tc.tile_pool(name="acts", bufs=3) as acts,
            tc.tile_pool(name="cmacts", bufs=5) as cmacts,
            tc.tile_pool(name="tiny", bufs=3) as tiny,
            tc.tile_pool(name="ptr", bufs=3, space="PSUM") as ptr,
            tc.tile_pool(name="pmm", bufs=3, space="PSUM") as pmm,
            tc.tile_pool(name="pz", bufs=2, space="PSUM") as pz,
            tc.tile_pool(name="dram", bufs=2, space="DRAM") as dram,
        ):
            # ---------------- collective warmup (absorbs first-call cost) ----
            warm_i = dram.tile([1, P], F32)
            warm_o = dram.tile([1, P], F32)
            wtile = tiny.tile([1, P], F32, tag="warm", bufs=1)
            nc.vector.memset(wtile[:], 0.0)
            nc.gpsimd.dma_start(warm_i[:], wtile[:])
            nc.gpsimd.collective_compute(
                "AllReduce", OP.add,
                replica_groups=[list(range(NCORES))],
                ins=[warm_i[:].opt()], outs=[warm_o[:].opt()],
            )

            # ---------------- constants ----------------
            ident = consts.tile([P, P], F32)
            nc.sync.dma_start(ident[:], ident_in[:])
            identb = consts.tile([P, P], BF)
            nc.sync.dma_start(identb[:], identb_in[:])
            ones = consts.tile([P, 1], F32)
            nc.sync.dma_start(ones[:], ones_in[:])
            onesb = consts.tile([P, 1], BF)
            nc.sync.dma_start(onesb[:], onesb_in[:])
            sel = consts.tile([P, S], F32)
            nc.sync.dma_start(sel[:], sel_in[:])
            eps_t = consts.tile([P, 1], F32)
            nc.vector.memset(eps_t[:], EPS)

            def cvec(dr, w):
                t = consts.tile([P, w], F32, name=dr.name + "_sb")
                nc.sync.dma_start(t[:], dr[:])
                return t

            bn1g_t = cvec(bn1_g, CT)
            bn1b_t = cvec(bn1_b, CT)
            lncag_t = cvec(lncag, CT)
            lncab_t = cvec(lncab, CT)
            saob_t = cvec(saob, CT)
            caob_t = cvec(caob, CT)
            ln1g_t = cvec(ln1g, CT)
            ln1b_t = cvec(ln1b, CT)
            ln2g_t = cvec(ln2g, MT)
            ln2b_t = cvec(ln2b, MT)
            bn2g_t = cvec(bn2_g, S)
            bn2b_t = cvec(bn2_b, S)

            def wload(dr, kt, width):
                t = wts.tile([P, kt, width], BF, name=dr.name + "_sb", tag="w512")
                nc.sync.dma_start(
                    t[:], dr[:].rearrange("(kt p) o -> p kt o", p=P)
                )
                return t

            # transpose helper -> psum tile (caller consumes psum directly)
            def tr_ps(src_ap, pin, n):
                dt = src_ap.dtype
                ps = ptr.tile([P, P], dt, tag="tr", name="tps")
                idt = (identb if dt == BF else ident)[:pin, :pin]
                nc.tensor.transpose(ps[:n, :pin], src_ap, idt)
                return ps

            # ---------------- queries load + BN1 partials ----------------
            qrm = acts.tile([P, 2, C], F32, tag="rm512")  # rows (4b x 32)
            nc.any.memzero(qrm[:])
            for b in range(BL):
                nc.sync.dma_start(
                    qrm[(b % 4) * SP:(b % 4) * SP + S, b // 4, :], q_in[b]
                )
            # full queries (host zero-padded to 2048 rows), local BN1 stats
            ps_s1 = ptr.tile([1, C], F32, tag="tr", name="ps_s1")
            ps_s2 = ptr.tile([1, C], F32, tag="tr", name="ps_s2")
            for chunk in range(4):
                qc = acts.tile([P, 4, C], F32, tag="qfl", bufs=2, name="qc")
                nc.sync.dma_start(
                    qc[:],
                    qall_in[chunk * 4 * P:(chunk + 1) * 4 * P, :]
                    .rearrange("(n p) c -> p n c", p=P))
                for i in range(4):
                    n = chunk * 4 + i
                    sq = acts.tile([P, C], F32, tag="qsq", bufs=2, name="sq")
                    nc.vector.tensor_mul(sq[:], qc[:, i], qc[:, i])
                    nc.tensor.matmul(ps_s1[:], ones[:], qc[:, i],
                                     start=(n == 0), stop=(n == 15))
                    nc.tensor.matmul(ps_s2[:], ones[:], sq[:],
                                     start=(n == 0), stop=(n == 15))
            st1 = tiny.tile([1, 2, C], F32, tag="st1", bufs=1)
            nc.vector.tensor_copy(st1[:, 0, :], ps_s1[:])
            nc.vector.tensor_copy(st1[:, 1, :], ps_s2[:])
            stats1 = tiny.tile([P, CT, 2], F32, tag="stats1")
            # bounce through DRAM to re-lay [1, 2C] row onto [p, ct, 2]
            bb1 = dram.tile([1, 2 * C], F32)
            nc.sync.dma_start(bb1[:], st1[:].rearrange("a b c -> a (b c)"))
            for j in range(2):
                nc.sync.dma_start(
                    stats1[:, :, j],
                    bb1[0, j * C:(j + 1) * C].rearrange("(ct p) -> p ct", p=P)
                )
            mu1 = tiny.tile([P, CT], F32, tag="mu1")
            nc.scalar.mul(mu1[:], stats1[:, :, 0], 1.0 / NROWS)
            ex2 = tiny.tile([P, CT], F32, tag="ex2")
            nc.scalar.mul(ex2[:], stats1[:, :, 1], 1.0 / NROWS)
            var1 = tiny.tile([P, CT], F32, tag="var1")
            nc.vector.tensor_mul(var1[:], mu1[:], mu1[:])
            nc.vector.tensor_sub(var1[:], ex2[:], var1[:])
            rstd1 = tiny.tile([P, CT], F32, tag="rstd1")
            nc.scalar.activation(rstd1[:], var1[:], AF.Sqrt,
                                 bias=eps_t[:], scale=1.0)
            nc.vector.reciprocal(rstd1[:], rstd1[:])
            s1 = tiny.tile([P, CT], F32, tag="s1")
            nc.vector.tensor_mul(s1[:], rstd1[:], bn1g_t[:])
            t1 = tiny.tile([P, CT], F32, tag="t1")
            nc.vector.tensor_mul(t1[:], mu1[:], s1[:])
            nc.vector.tensor_sub(t1[:], bn1b_t[:], t1[:])

            # ---------------- xnT (bf16, fused BN1 affine) ----------------
            xnT = cmacts.tile([P, CT, RWS], BF, tag="cmb")
            for rt in range(2):
                for ck in range(CT):
                    ps = tr_ps(qrm[:, rt, ck * P:(ck + 1) * P], P, P)
                    nc.vector.tensor_scalar(
                        xnT[:, ck, rt * P:(rt + 1) * P], ps[:],
                        scalar1=s1[:, ck:ck + 1], scalar2=t1[:, ck:ck + 1],
                        op0=OP.mult, op1=OP.add)

            # ---------------- SA projections (bf16, channel-major) --------
            w_q = wload(w_saqT, CT, C)
            w_k = wload(w_sakT, CT, C)
            w_v = wload(w_savT, CT, C)

            def proj(wt, src, kt_n, name):
                out = cmacts.tile([P, CT, RWS], BF, tag="cmb", name=name)
                for ot in range(CT):
                    ps = pmm.tile([P, RWS], F32, tag="mm", name="ps_" + name)
                    for kt in range(kt_n):
                        nc.tensor.matmul(
                            ps[:], wt[:, kt, ot * P:(ot + 1) * P], src[:, kt],
                            start=(kt == 0), stop=(kt == kt_n - 1))
                    nc.any.tensor_copy(out[:, ot], ps[:])
                return out

            qcm = proj(w_q, xnT, CT, "qcm")
            kcm = proj(w_k, xnT, CT, "kcm")
            vcm = proj(w_v, xnT, CT, "vcm")

            # ---------------- SA energy per (b,h) ----------------
            e_rm = acts.tile([P, 16, S], F32, tag="ebh", bufs=2)
            nc.any.memzero(e_rm[:])
            for b in range(BL):
                for h in range(H):
                    g = (b * H + h) // 4
                    r0 = ((b * H + h) % 4) * SP
                    off = (h % 2) * DH
                    ht = h // 2
                    pse = ptr.tile([S, S], F32, tag="tr", name="ps_esa")
                    nc.tensor.matmul(
                        pse[:],
                        qcm[off:off + DH, ht, b * SP:b * SP + S],
                        kcm[off:off + DH, ht, b * SP:b * SP + S],
                        start=True, stop=True,
                        tile_position=(off, 0))
                    nc.any.tensor_copy(e_rm[r0:r0 + S, g, :], pse[:])

            # ---------------- BN2 partials + AllReduce ----------------
            esq = acts.tile([P, 16, S], F32, tag="ebh", bufs=2)
            for g in range(16):
                nc.vector.tensor_mul(esq[:, g], e_rm[:, g], e_rm[:, g])
            ps_b1 = ptr.tile([S, S], F32, tag="tr", name="ps_b1")
            ps_b2 = ptr.tile([S, S], F32, tag="tr", name="ps_b2")
            for g in range(16):
                nc.tensor.matmul(ps_b1[:], sel[:], e_rm[:, g],
                                 start=(g == 0), stop=(g == 15))
            for g in range(16):
                nc.tensor.matmul(ps_b2[:], sel[:], esq[:, g],
                                 start=(g == 0), stop=(g == 15))
            st2 = tiny.tile([S, 2, S], F32, tag="st2", bufs=1)
            nc.any.tensor_copy(st2[:, 0, :], ps_b1[:])
            nc.any.tensor_copy(st2[:, 1, :], ps_b2[:])
            ar2_i = dram.tile([S, 2 * S], F32)
            ar2_o = dram.tile([S, 2 * S], F32)
            nc.sync.dma_start(ar2_i[:], st2[:].rearrange("a b c -> a (b c)"))
            nc.gpsimd.collective_compute(
                "AllReduce", OP.add,
                replica_groups=[list(range(NCORES))],
                ins=[ar2_i[:].opt()], outs=[ar2_o[:].opt()],
            )
            stats2 = tiny.tile([P, 2, S], F32, tag="stats2")
            nc.any.memzero(stats2[:])
            for rep in range(4):
                nc.sync.dma_start(
                    stats2[rep * SP:rep * SP + S],
                    ar2_o[:].rearrange("s (j sp) -> s j sp", j=2))
            mu2 = tiny.tile([P, S], F32, tag="mu2")
            nc.scalar.mul(mu2[:], stats2[:, 0], 1.0 / NBH)
            ex22 = tiny.tile([P, S], F32, tag="ex22")
            nc.scalar.mul(ex22[:], stats2[:, 1], 1.0 / NBH)
            var2 = tiny.tile([P, S], F32, tag="var2")
            nc.vector.tensor_mul(var2[:], mu2[:], mu2[:])
            nc.vector.tensor_sub(var2[:], ex22[:], var2[:])
            rstd2 = tiny.tile([P, S], F32, tag="rstd2")
            nc.scalar.activation(rstd2[:], var2[:], AF.Sqrt,
                                 bias=eps_t[:], scale=1.0)
            nc.vector.reciprocal(rstd2[:], rstd2[:])
            sc2 = tiny.tile([P, S], F32, tag="sc2")
            nc.vector.tensor_mul(sc2[:], rstd2[:], bn2g_t[:])
            off2 = tiny.tile([P, S], F32, tag="off2")
            nc.vector.tensor_mul(off2[:], mu2[:], sc2[:])
            nc.vector.tensor_sub(off2[:], bn2b_t[:], off2[:])

            # normalize + softmax (over s', free dim); attn cast to bf16
            attn_bf = acts.tile([P, 16, S], BF, tag="abh", bufs=1)
            rs_sa = tiny.tile([P, 16], F32, tag="rs_sa")
            for g in range(16):
                nc.vector.tensor_mul(e_rm[:, g], e_rm[:, g], sc2[:])
                nc.vector.tensor_add(e_rm[:, g], e_rm[:, g], off2[:])
                nc.scalar.activation(e_rm[:, g], e_rm[:, g], AF.Exp,
                                     bias=0.0, scale=INV_SCALE)
                nc.vector.reduce_sum(rs_sa[:, g:g + 1], e_rm[:, g],
                                     axis=mybir.AxisListType.X)
            nc.vector.reciprocal(rs_sa[:], rs_sa[:])
            for g in range(16):
                nc.vector.tensor_scalar_mul(attn_bf[:, g], e_rm[:, g],
                                            rs_sa[:, g:g + 1])

            # vT (bf16, row-major)
            vT = acts.tile([P, 2, C], BF, tag="rmb512", bufs=2, name="vT")
            for rt in range(2):
                for ot in range(CT):
                    ps = tr_ps(vcm[:, ot, rt * P:(rt + 1) * P], P, P)
                    nc.any.tensor_copy(vT[:, rt, ot * P:(ot + 1) * P], ps[:])

            # attn.T then attn @ v -> o row-major (fp32)
            o_rm = acts.tile([P, 2, C], F32, tag="rm512", name="o_rm")
            nc.any.memzero(o_rm[:])
            for g in range(16):
                b = g // 2
                po = (b % 4) * SP
                at_t = acts.tile([P, P], BF, tag="atb", bufs=4, name="at_t")
                ps = tr_ps(attn_bf[:, g, :], P, S)
                nc.any.tensor_copy(at_t[po:po + S, :], ps[:S, :])
                for j in range(4):
                    h = (g * 4 + j) % H
                    pso = ptr.tile([S, DH], F32, tag="tr", name="ps_o")
                    nc.tensor.matmul(
                        pso[:],
                        at_t[po:po + S, j * SP:j * SP + S],
                        vT[po:po + S, b // 4, h * DH:(h + 1) * DH],
                        start=True, stop=True,
                        tile_position=(po, 0))
                    nc.any.tensor_copy(
                        o_rm[po:po + S, b // 4, h * DH:(h + 1) * DH], pso[:])

            # ---------------- SA out proj + residual ----------------
            w_o = wload(w_saoT, CT, C)
            oT = cmacts.tile([P, CT, RWS], BF, tag="cmb", name="oT")
            for rt in range(2):
                for ck in range(CT):
                    ps = tr_ps(o_rm[:, rt, ck * P:(ck + 1) * P], P, P)
                    nc.any.tensor_copy(oT[:, ck, rt * P:(rt + 1) * P], ps[:])
            xo_cm = cmacts.tile([P, CT, RWS], F32, tag="cm32", bufs=2,
                                name="xo_cm")
            for ot in range(CT):
                ps = pmm.tile([P, RWS], F32, tag="mm", name="ps_xo")
                for kt in range(CT):
                    nc.tensor.matmul(
                        ps[:], w_o[:, kt, ot * P:(ot + 1) * P], oT[:, kt],
                        start=(kt == 0), stop=(kt == CT - 1))
                nc.vector.tensor_scalar_add(
                    xo_cm[:, ot], ps[:], saob_t[:, ot:ot + 1])
            xsa = acts.tile([P, 2, C], F32, tag="rm512", name="xsa")
            for rt in range(2):
                for ck in range(CT):
                    ps = tr_ps(xo_cm[:, ck, rt * P:(rt + 1) * P], P, P)
                    nc.vector.tensor_add(xsa[:, rt, ck * P:(ck + 1) * P],
                                         ps[:], qrm[:, rt, ck * P:(ck + 1) * P])

            # ---------------- CA: LN(x_sa) -> q2 -> A ----------------
            def ln_rows(src, width, nst, name, inplace=None):
                """row-wise layernorm core (x-mu)*rstd, no affine (fp32)"""
                if inplace is not None:
                    out = inplace
                else:
                    out = acts.tile([P, 2, width], F32,
                                    tag=("rm512" if width == C else "rm1024"),
                                    bufs=(None if width == C else 1),
                                    name=name)
                for rt in range(2):
                    stt = tiny.tile([P, nst, 6], F32, tag="bnst")
                    for j in range(nst):
                        nc.vector.bn_stats(stt[:, j],
                                           src[:, rt, j * C:(j + 1) * C])
                    mv = tiny.tile([P, 2], F32, tag="bnmv")
                    nc.vector.bn_aggr(mv[:], stt[:])
                    rst = tiny.tile([P, 1], F32, tag="bnrs")
                    nc.scalar.activation(rst[:], mv[:, 1:2], AF.Sqrt,
                                         bias=eps_t[:], scale=1.0)
                    nc.vector.reciprocal(rst[:], rst[:])
                    nc.vector.tensor_scalar(
                        out[:, rt], src[:, rt],
                        scalar1=mv[:, 0:1], scalar2=rst[:],
                        op0=OP.subtract, op1=OP.mult)
                return out

            qn = ln_rows(xsa, C, 1, "qn")
            qnT = cmacts.tile([P, CT, RWS], BF, tag="cmb", name="qnT")
            for rt in range(2):
                for ck in range(CT):
                    ps = tr_ps(qn[:, rt, ck * P:(ck + 1) * P], P, P)
                    nc.vector.tensor_scalar(
                        qnT[:, ck, rt * P:(rt + 1) * P], ps[:],
                        scalar1=lncag_t[:, ck:ck + 1],
                        scalar2=lncab_t[:, ck:ck + 1],
                        op0=OP.mult, op1=OP.add)
            w_q2 = wload(w_caqT, CT, C)
            q2cm = proj(w_q2, qnT, CT, "q2cm")

            # A_bf[c, (ct,h,rows)] bf16
            w_k2 = wload(w_cak, CT, C)
            A_bf = big.tile([P, CT, H, RWS], BF, tag="big32", name="A_bf")
            for h in range(H):
                off = (h % 2) * DH
                ht = h // 2
                for ck in range(CT):
                    ps = pmm.tile([P, RWS], F32, tag="mm", name="ps_a")
                    nc.tensor.matmul(
                        ps[:],
                        w_k2[off:off + DH, ht, ck * P:(ck + 1) * P],
                        q2cm[off:off + DH, ht, :],
                        start=True, stop=True,
                        tile_position=(off, 0))
                    nc.any.tensor_copy(A_bf[:, ck, h, :], ps[:])

            # ---------------- main ctx loop ----------------
            ZT = big.tile([P, CT, H, RWS], BF, tag="big32", name="ZT")
            nc.any.memzero(ZT[:])
            for b in range(BL):
                # ctx.T for the whole batch via xbar DMA transpose
                cT = ctxtp.tile([P, CT, CTX], BF, tag="ct", name="cT")
                for ck in range(CT):
                    nc.sync.dma_start_transpose(
                        cT[:, ck, :], ctx_in[b, :, ck * P:(ck + 1) * P])
                p_t = ptp.tile([P, TT, H * S], BF, tag="pt", name="p_t")
                ps_z = [pz.tile([124, C], F32, tag="z", name=f"ps_z{half}")
                        for half in range(2)]
                rsum = tiny.tile([1, H * S], F32, tag="rsum", bufs=2)
                for quarter in range(4):
                    cnat = ctxp.tile([P, 4, C], BF, tag="cn", name="cnat")
                    nc.gpsimd.dma_start(
                        cnat[:],
                        ctx_in[b, quarter * 4 * P:(quarter + 1) * 4 * P, :]
                        .rearrange("(i p) c -> p i c", p=P))
                    e_buf = ebp.tile([P, 4, H, S], F32, tag="eb",
                                     name="e_buf")
                    ps_row = ptr.tile([1, H * S], F32, tag="tr",
                                      name="ps_row")
                    for i in range(4):
                        tt = quarter * 4 + i
                        ps_e = pmm.tile([P, RWS], F32, tag="mm", name="ps_e")
                        for ck in range(CT):
                            nc.tensor.matmul(
                                ps_e[:],
                                cT[:, ck, tt * P:(tt + 1) * P],
                                A_bf[:, ck, :, b * SP:(b + 1) * SP],
                                start=(ck == 0), stop=(ck == CT - 1))
                        pse_v = ps_e[:].rearrange("p (h s) -> p h s", h=H)
                        nc.vector.tensor_copy(e_buf[:, i], pse_v[:, :, :S])
                        nc.scalar.activation(
                            p_t[:, tt].rearrange("p (h s) -> p h s", h=H),
                            pse_v[:, :, :S],
                            AF.Exp, bias=0.0, scale=INV_SCALE)
                        nc.tensor.matmul(
                            ps_row[:], onesb[:], p_t[:, tt, :],
                            start=(i == 0), stop=(i == 3))
                        for half in range(2):
                            nc.tensor.matmul(
                                ps_z[half],
                                p_t[:, tt, half * 124:(half + 1) * 124],
                                cnat[:, i],
                                start=(tt == 0), stop=(tt == TT - 1))
                    if quarter == 0:
                        nc.vector.tensor_copy(rsum[:], ps_row[:])
                    else:
                        nc.vector.tensor_add(rsum[:], rsum[:], ps_row[:])
                    nc.gpsimd.dma_start(
                        e_out[b, quarter * 4 * P:(quarter + 1) * 4 * P, :]
                        .rearrange("(a p) x -> p a x", p=P),
                        e_buf[:].rearrange("p a h s -> p a (h s)"))
                # softmax denominators -> [124, 2] reciprocals
                rr = tiny.tile([124, 2], F32, tag="rr", bufs=2)
                for half in range(2):
                    pst = ptr.tile([124, 1], F32, tag="tr", name="ps_t")
                    nc.tensor.matmul(
                        pst[:], rsum[:1, half * 124:(half + 1) * 124],
                        ident[:1, :1], start=True, stop=True)
                    nc.any.tensor_copy(rr[:, half:half + 1], pst[:])
                rinv = tiny.tile([124, 2], F32, tag="rinv", bufs=2)
                nc.vector.reciprocal(rinv[:], rr[:])
                for half in range(2):
                    z_bf = acts.tile([124, C], BF, tag="zsb", bufs=2,
                                     name="z_bf")
                    nc.vector.tensor_scalar_mul(z_bf[:], ps_z[half][:],
                                                rinv[:, half:half + 1])
                    for ck in range(CT):
                        ps = ptr.tile([P, P], BF, tag="tr", name="tps_z")
                        nc.tensor.transpose(ps[:, :124],
                                            z_bf[:, ck * P:(ck + 1) * P],
                                            identb[:124, :124])
                        nc.vector.tensor_copy(
                            ZT[:, ck, half * 4:(half + 1) * 4,
                               b * SP:b * SP + S],
                            ps[:, :124].rearrange("p (h s) -> p h s", s=S))

            # ---------------- o2 / out2 / residual ----------------
            w_v2 = wload(w_cavT, CT, C)
            o2T = cmacts.tile([P, CT, RWS], BF, tag="cmb", name="o2T")
            nc.any.memzero(o2T[:])
            for h in range(H):
                off = (h % 2) * DH
                ht = h // 2
                ps = pmm.tile([P, RWS], F32, tag="mm", name="ps_o2")
                for ck in range(CT):
                    nc.tensor.matmul(
                        ps[:DH, :],
                        w_v2[:, ck, h * DH:(h + 1) * DH],
                        ZT[:, ck, h, :],
                        start=(ck == 0), stop=(ck == CT - 1))
                nc.any.tensor_copy(o2T[off:off + DH, ht, :], ps[:DH, :])
            w_o2 = wload(w_caoT, CT, C)
            xca = acts.tile([P, 2, C], F32, tag="rm512", name="xca")
            out2cm = cmacts.tile([P, CT, RWS], F32, tag="cm32", bufs=2,
                                 name="out2cm")
            for ot in range(CT):
                ps = pmm.tile([P, RWS], F32, tag="mm", name="ps_out2")
                for kt in range(CT):
                    nc.tensor.matmul(
                        ps[:], w_o2[:, kt, ot * P:(ot + 1) * P], o2T[:, kt],
                        start=(kt == 0), stop=(kt == CT - 1))
                nc.vector.tensor_scalar_add(
                    out2cm[:, ot], ps[:], caob_t[:, ot:ot + 1])
            for rt in range(2):
                for ck in range(CT):
                    ps = tr_ps(out2cm[:, ck, rt * P:(rt + 1) * P], P, P)
                    nc.vector.tensor_add(xca[:, rt, ck * P:(ck + 1) * P],
                                         ps[:], xsa[:, rt, ck * P:(ck + 1) * P])

            # ---------------- FFN ----------------
            h1 = ln_rows(xca, C, 1, "h1")
            h1T = cmacts.tile([P, CT, RWS], BF, tag="cmb", name="h1T")
            for rt in range(2):
                for ck in range(CT):
                    ps = tr_ps(h1[:, rt, ck * P:(ck + 1) * P], P, P)
                    nc.scalar.activation(
                        h1T[:, ck, rt * P:(rt + 1) * P], ps[:], AF.Silu,
                        bias=ln1b_t[:, ck:ck + 1], scale=ln1g_t[:, ck:ck + 1])
            midcm = cmacts.tile([P, MT, RWS], F32, tag="cm32x8", bufs=1,
                                name="midcm")
            for hf in range(2):
                w_f1 = wts.tile([P, CT, C], BF, tag="w512", name=f"w_f1{hf}")
                nc.sync.dma_start(
                    w_f1[:],
                    w_ff1T[:, hf * C:(hf + 1) * C]
                    .rearrange("(kt p) o -> p kt o", p=P))
                for mt_l in range(CT):
                    mt = hf * CT + mt_l
                    ps = pmm.tile([P, RWS], F32, tag="mm", name="ps_mid")
                    for kt in range(CT):
                        nc.tensor.matmul(
                            ps[:], w_f1[:, kt, mt_l * P:(mt_l + 1) * P],
                            h1T[:, kt],
                            start=(kt == 0), stop=(kt == CT - 1))
                    nc.any.tensor_copy(midcm[:, mt], ps[:])
            h2 = acts.tile([P, 2, MID], F32, tag="rm1024", bufs=1, name="h2")
            for rt in range(2):
                for mk in range(MT):
                    ps = tr_ps(midcm[:, mk, rt * P:(rt + 1) * P], P, P)
                    nc.any.tensor_copy(h2[:, rt, mk * P:(mk + 1) * P], ps[:])
            h2n = ln_rows(h2, MID, 2, "h2n", inplace=h2)
            h2T = cmacts.tile([P, MT, RWS], BF, tag="cmbx8", bufs=1,
                              name="h2T")
            for rt in range(2):
                for mk in range(MT):
                    ps = tr_ps(h2n[:, rt, mk * P:(mk + 1) * P], P, P)
                    nc.scalar.activation(
                        h2T[:, mk, rt * P:(rt + 1) * P], ps[:], AF.Silu,
                        bias=ln2b_t[:, mk:mk + 1], scale=ln2g_t[:, mk:mk + 1])
            outx = acts.tile([P, 2, C], F32, tag="rm512", name="outx")
            ffcm = cmacts.tile([P, CT, RWS], F32, tag="cm32", bufs=2,
                               name="ffcm")
            w_f2h = []
            for hf in range(2):
                t = wts.tile([P, CT, C], BF, tag="w512", name=f"w_f2{hf}")
                nc.sync.dma_start(
                    t[:],
                    w_ff2T[hf * C:(hf + 1) * C, :]
                    .rearrange("(kt p) o -> p kt o", p=P))
                w_f2h.append(t)
            for ot in range(CT):
                ps = pmm.tile([P, RWS], F32, tag="mm", name="ps_ff")
                for kt in range(MT):
                    nc.tensor.matmul(
                        ps[:],
                        w_f2h[kt // CT][:, kt % CT, ot * P:(ot + 1) * P],
                        h2T[:, kt],
                        start=(kt == 0), stop=(kt == MT - 1))
                nc.any.tensor_copy(ffcm[:, ot], ps[:])
            for rt in range(2):
                for ck in range(CT):
                    ps = tr_ps(ffcm[:, ck, rt * P:(rt + 1) * P], P, P)
                    nc.vector.tensor_add(outx[:, rt, ck * P:(ck + 1) * P],
                                         ps[:], xca[:, rt, ck * P:(ck + 1) * P])
            for b in range(BL):
                nc.gpsimd.dma_start(
                    out_x[b], outx[(b % 4) * SP:(b % 4) * SP + S, b // 4, :])

    nc.compile()
    return nc


def host_inputs(inputs):
    """Build per-core input maps from full inputs."""
    import ml_dtypes
    f = np.float32
    bf = ml_dtypes.bfloat16
    q = np.asarray(inputs["queries"], f)
    ctx = np.asarray(inputs["contexts"], f)

    def cm(v):  # [C] -> [P, C//P] channel-major per-partition layout
        v = np.asarray(v, f).reshape(-1, P)
        return np.ascontiguousarray(v.T)

    def tb(v):  # transpose + bf16
        return np.ascontiguousarray(np.asarray(v, f).T).astype(bf)

    shared = {
        "w_saqT": tb(inputs["sa_q_w"]),
        "w_sakT": tb(inputs["sa_k_w"]),
        "w_savT": tb(inputs["sa_v_w"]),
        "w_saoT": tb(inputs["sa_out_w"]),
        "w_caqT": tb(inputs["ca_q_w"]),
        "w_cak": np.asarray(inputs["ca_k_w"], f).astype(bf),
        "w_cavT": tb(inputs["ca_v_w"]),
        "w_caoT": tb(inputs["ca_out_w"]),
        "w_ff1T": tb(inputs["ff_w1"]),
        "w_ff2T": tb(inputs["ff_w2"]),
        "bn1_g": cm(inputs["sa_bn_g"]),
        "bn1_b": cm(inputs["sa_bn_b"]),
        "bn2_g": np.tile(np.vstack([np.asarray(inputs["sa_pre_g"], f)
                                    .reshape(S, S), np.zeros((1, S), f)]), (4, 1)),
        "bn2_b": np.tile(np.vstack([np.asarray(inputs["sa_pre_b"], f)
                                    .reshape(S, S), np.zeros((1, S), f)]), (4, 1)),
        "lncag": cm(inputs["ca_ln_g"]),
        "lncab": cm(inputs["ca_ln_b"]),
        "saob": cm(inputs["sa_out_b"]),
        "caob": cm(inputs["ca_out_b"]),
        "ln1g": cm(inputs["ff_ln1_g"]),
        "ln1b": cm(inputs["ff_ln1_b"]),
        "ln2g": cm(inputs["ff_ln2_g"]),
        "ln2b": cm(inputs["ff_ln2_b"]),
        "ident": np.eye(P, dtype=f),
        "identb": np.eye(P, dtype=f).astype(bf),
        "ones": np.ones((P, 1), f),
        "onesb": np.ones((P, 1), f).astype(bf),
        "sel": np.tile(np.vstack([np.eye(S, dtype=f),
                                  np.zeros((1, S), f)]), (4, 1)),
    }
    ctx_bf = ctx.astype(bf)
    q_flat = np.zeros((2048, C), f)
    q_flat[:B * S] = q.reshape(B * S, C)
    in_maps = []
    for i in range(NCORES):
        m = dict(shared)
        m["q_in"] = np.ascontiguousarray(q[i * BL:(i + 1) * BL])
        m["qall_in"] = q_flat
        m["ctx_in"] = np.ascontiguousarray(ctx_bf[i * BL:(i + 1) * BL])
        in_maps.append(m)
    return in_maps


_CACHE = {}


def _get_nc():
    if "nc" not in _CACHE:
        _CACHE["nc"] = build_bass()
    return _CACHE["nc"]


def run(inputs, trace=False, **kw):
    from concourse.bass_utils import run_bass_kernel_spmd

    nc = _get_nc()
    in_maps = host_inputs(inputs)
    res = run_bass_kernel_spmd(nc, in_maps, core_ids=list(range(NCORES)),
                               trace=trace, **kw)
    outs = []
    es = []
    for i in range(NCORES):
        outs.append(res.results[i]["out_x"])
        es.append(res.results[i]["e_out"])
    out = np.concatenate(outs, axis=0).astype(np.float32)
    e_t = np.concatenate(es, axis=0)  # [B, CTX, H*S]
    energy2 = np.ascontiguousarray(
        e_t.reshape(B, CTX, H, S).transpose(0, 2, 3, 1)).astype(np.float32)
    return (out, energy2), res


def kernel(**inputs):
    (out, energy2), _ = run(inputs)
    return out, energy2
